# revision 44
# baseline (speedup 1.0000x reference)
"""Trainium2 Bass kernel for nn_Decoder_Model (dense transformer decoder layer).

Sharding: data-parallel over batch (8 batches -> 8 cores). The three global
layernorms (normalized over ALL elements of the [B,S,D] tensor) need cross-core
scalar stats: each core computes local sum/sumsq, an 8-float AllReduce merges
them. AllReduce latency is hidden behind the next phase's matmuls using the
affine trick: norm(x)@W.T = (x@W.T)*rstd + per-channel-fix, so the big matmuls
run on raw x while stats are in flight and only a cheap fixup pass waits.

Compute is bf16 (tolerance 2e-2; measured ~2e-3): matmuls run at full PE rate,
f32 PSUM accumulation. All weight/activation transposes go through the DMA
xbar (bf16 2-byte path), so the tensor engine runs a pure matmul stream.
Causal self-attn scores/PV only compute the valid lower-triangle chunks; the
softmax denominator rides along as a ones-column appended to V, and attention
normalization is deferred: raw PV outputs are staged, denominator rows are
batched into one [16,512] reciprocal, then scaled in place.
"""
import sys

import numpy as np

sys.path.insert(0, "/opt/trn_rl_repo")

import concourse.bass as bass  # noqa: E402,F401
import concourse.mybir as mybir  # noqa: E402
import concourse.tile as tile  # noqa: E402
from concourse import bacc  # noqa: E402
from concourse import bass_utils  # noqa: E402

F32 = mybir.dt.float32
BF16 = mybir.dt.bfloat16
AF = mybir.ActivationFunctionType
OP = mybir.AluOpType

B, S, D, H, DK, FF = 8, 1024, 512, 8, 64, 2048
TT = S // 128   # 8 token tiles
DT = D // 128   # 4 feature tiles
FT = FF // 128  # 16 ffn tiles
TH = S // 512   # 2 matmul free-dim halves
N_CORES = 8
NTOT = float(B * S * D)
EPS = 1e-5

WNAMES = ["wq_m", "wk_m", "wv_m", "wo_m", "wq_c", "wk_c", "wv_c", "wo_c"]
BNAMES = ["bq_m", "bk_m", "bv_m", "bo_m", "bq_c", "bk_c", "bv_c", "bo_c"]


def build_nc(collectives=True):
    nc = bacc.Bacc("TRN2", target_bir_lowering=False, debug=False,
                   enable_asserts=False,
                   num_devices=N_CORES if collectives else 1)
    inp = {}
    inp["data_dec"] = nc.dram_tensor("data_dec", [S, D], F32,
                                     kind="ExternalInput").ap()
    inp["encoder_out"] = nc.dram_tensor("encoder_out", [S, D], F32,
                                        kind="ExternalInput").ap()
    for w in WNAMES:
        inp[w] = nc.dram_tensor(w, [D, D], F32, kind="ExternalInput").ap()
    for b in BNAMES:
        inp[b] = nc.dram_tensor(b, [D], F32, kind="ExternalInput").ap()
    inp["wf1"] = nc.dram_tensor("wf1", [FF, D], F32, kind="ExternalInput").ap()
    inp["bf1"] = nc.dram_tensor("bf1", [FF], F32, kind="ExternalInput").ap()
    inp["wf2"] = nc.dram_tensor("wf2", [D, FF], F32, kind="ExternalInput").ap()
    inp["bf2"] = nc.dram_tensor("bf2", [D], F32, kind="ExternalInput").ap()
    out_d = nc.dram_tensor("out", [S, D], F32, kind="ExternalOutput").ap()

    with tile.TileContext(nc) as tc:
        build_body(nc, tc, inp, out_d, collectives)
    nc.finalize()
    return nc


def build_body(nc, tc, inp, out_d, collectives=True):
    import contextlib
    ctx = contextlib.ExitStack()
    with ctx:
        sb = ctx.enter_context(tc.tile_pool(name="sb", bufs=1))
        stg = ctx.enter_context(tc.tile_pool(name="stg", bufs=2))
        stgb = ctx.enter_context(tc.tile_pool(name="stgb", bufs=2))
        scr = ctx.enter_context(tc.tile_pool(name="scr", bufs=2))
        cp = ctx.enter_context(tc.tile_pool(name="cp", bufs=3))
        dram = ctx.enter_context(tc.tile_pool(name="dram", bufs=1, space="DRAM"))
        ps_sc = ctx.enter_context(tc.tile_pool(name="ps_sc", bufs=2, space="PSUM"))
        ps_b = ctx.enter_context(tc.tile_pool(name="ps_b", bufs=4, space="PSUM"))

        # ---------------- weight prep: load f32 -> cast bf16 -> DMA-xbar
        # transpose. Attn weights land as [128(din), 16, 128] with block
        # index dd*4+ki (one xbar call per weight); wv keeps [128, ki, 512]
        # (dout-contiguous) for the token-major v projection. ------------
        def stage_cast(src_ap, engine="dve"):
            """DMA a [128 x 2048-elem] f32 chunk and cast to bf16."""
            st = stg.tile([128, 2048], F32, tag="wstage", name="wst")
            nc.sync.dma_start(st[:], src_ap)
            bst = stgb.tile([128, 2048], BF16, tag="wstage_bf", name="wstb")
            if engine == "dve":
                nc.vector.tensor_copy(bst[:], st[:])
            else:
                nc.scalar.copy(bst[:], st[:])
            return bst

        wT = {}

        def prep_w(name, colsum_to=None, engine="dve", defer=False):
            """512x512 weight -> wT[name] [128, 16, 128], block = dd*4+ki."""
            bst = stage_cast(inp[name].rearrange("(t p) i -> p t i", p=128),
                             engine)
            if colsum_to is not None:
                for t in range(DT):
                    nc.vector.reduce_sum(colsum_to[:, t:t + 1],
                                         bst[:, t * 512:(t + 1) * 512],
                                         axis=mybir.AxisListType.X)
            w = sb.tile([128, 16, 128], BF16, tag=f"wT_{name}")

            def fin():
                nc.sync.dma_start(w[:], bst[:], transpose=True)
                wT[name] = w
            if defer:
                return fin
            fin()
            return w

        def prep_wv(name, engine="dve"):
            """512x512 weight -> wT[name] [128, DT(ki), 512] (dout-contig)."""
            bst = stage_cast(inp[name].rearrange("(t p) i -> p t i", p=128),
                             engine)
            w = sb.tile([128, DT, 512], BF16, tag=f"wT_{name}")
            for t in range(DT):
                nc.sync.dma_start(w[:, :, t * 128:(t + 1) * 128],
                                  bst[:, t * 512:(t + 1) * 512],
                                  transpose=True)
            wT[name] = w
            return w

        def prep_act(name, dst, engine="dve", defer=False):
            """[S, D] f32 activations -> dst [128, DT, S] bf16 feature-major.
            Loads ride the sync queue; casts+transposes the scalar queue."""
            fins = []
            for c in range(2):
                bst = stage_cast(
                    inp[name].rearrange("(tt p) d -> p tt d", p=128)
                    [:, 4 * c:4 * c + 4], engine)

                def fin(bst=bst, c=c):
                    for t in range(4):
                        tt = 4 * c + t
                        nc.scalar.dma_start(
                            dst[:, :, tt * 128:(tt + 1) * 128],
                            bst[:, t * 512:(t + 1) * 512], transpose=True)
                if defer:
                    fins.append(fin)
                else:
                    fin()
            if defer:
                return fins

        x_T = sb.tile([128, DT, S], BF16, tag="g_x")
        enc_T = sb.tile([128, DT, S], BF16, tag="g_enc")
        wsum_qc = sb.tile([128, DT], F32, tag="wsum_qc")
        wsum_f1 = sb.tile([128, FT], F32, tag="wsum_f1")
        wf1T = sb.tile([128, 64, 128], BF16, tag="wf1T")
        wf2T = sb.tile([128, 64, 128], BF16, tag="wf2T")

        def prep_wf(engine="act"):
            """wf1 -> [128,64,128] block ft*4+ki; wf2 -> block dd*16+ki."""
            for c in range(4):
                bst = stage_cast(
                    inp["wf1"].rearrange("(t p) i -> p t i", p=128)
                    [:, 4 * c:4 * c + 4], engine)
                for t in range(4):
                    rt = 4 * c + t
                    nc.vector.reduce_sum(wsum_f1[:, rt:rt + 1],
                                         bst[:, t * 512:(t + 1) * 512],
                                         axis=mybir.AxisListType.X)
                nc.sync.dma_start(wf1T[:, 16 * c:16 * (c + 1), :], bst[:],
                                  transpose=True)
            for rt in range(4):
                bst = stage_cast(
                    inp["wf2"].rearrange("(t p) i -> p t i", p=128)[:, rt],
                    engine)
                nc.sync.dma_start(wf2T[:, 16 * rt:16 * (rt + 1), :], bst[:],
                                  transpose=True)

        # phase-1-critical prep first: dispatch all loads back-to-back on the
        # sync queue BEFORE any transpose can park at a queue head, then emit
        # the transposes (scalar queue, ordered behind their casts)
        fq = prep_w("wq_m", defer=True)
        fx = prep_act("data_dec", x_T, defer=True)
        fq()
        for f in fx:
            f()
        prep_w("wk_m")
        prep_wv("wv_m")

        onesf = sb.tile([128, 1], F32, tag="onesf")
        nc.vector.memset(onesf[:], 1.0)
        # triangular causal mask for the diagonal block: keep q_col >= k_row
        mask_f = sb.tile([128, 128], F32, tag="mask_f")
        nc.vector.memset(mask_f[:], 1.0)
        nc.gpsimd.affine_select(out=mask_f[:], in_=mask_f[:],
                                compare_op=OP.is_ge, fill=0.0, base=0,
                                channel_multiplier=-1, pattern=[[1, 128]])
        mask_b = sb.tile([128, 128], BF16, tag="mask_b")
        nc.vector.tensor_copy(mask_b[:], mask_f[:])

        # ---------------- biases ----------------
        bias = {}
        for b in BNAMES + ["bf2"]:
            t = sb.tile([128, DT], F32, tag=f"{b}_sb")
            nc.sync.dma_start(t[:], inp[b].rearrange("(t p) -> p t", p=128))
            bias[b] = t
        bf1_sb = sb.tile([128, FT], F32, tag="bf1_sb")
        nc.sync.dma_start(bf1_sb[:], inp["bf1"].rearrange("(t p) -> p t", p=128))
        bv_full = {}
        for b in ("bv_m", "bv_c"):
            row = stg.tile([128, 2048], F32, tag="wstage", name="bvrow")
            nc.sync.dma_start(row[0:1, 0:D], inp[b][None, :])
            rowb = sb.tile([1, D], BF16, tag=f"{b}_rowb")
            nc.vector.tensor_copy(rowb[:], row[0:1, 0:D])
            full = sb.tile([128, D], BF16, tag=f"{b}_full")
            nc.gpsimd.partition_broadcast(full[:], rowb[:])
            bv_full[b] = full

        # ---------------- helpers ----------------
        def psB():
            return ps_b.tile([128, 512], F32, tag="B", name="pB")

        def project(wname, src_T, out_T, bname=None, fix=None, rstd=None):
            """out_T[:, dd, :] = act(W^T @ src_T) feature-major."""
            w = wT[wname]
            for dd in range(DT):
                for th in range(TH):
                    pt = psB()
                    for ki in range(DT):
                        nc.tensor.matmul(
                            pt[:], w[:, dd * 4 + ki, :],
                            src_T[:, ki, th * 512:(th + 1) * 512],
                            start=(ki == 0), stop=(ki == DT - 1))
                    dst = out_T[:, dd, th * 512:(th + 1) * 512]
                    if fix is not None:
                        nc.scalar.activation(dst, pt[:], AF.Identity,
                                             bias=fix[:, dd:dd + 1],
                                             scale=rstd[:])
                    else:
                        nc.vector.tensor_scalar_add(dst, pt[:],
                                                    bias[bname][:, dd:dd + 1])

        def project_v(wname, bname, src_T, v_tok):
            """Token-major v with per-head ones column: v_tok [128,TT,H*65]."""
            w = wT[wname]
            ones_view = v_tok[:, :, :].rearrange(
                "p t (h c) -> p t h c", c=65)[:, :, :, 64]
            nc.vector.tensor_copy(
                ones_view, onesf[:, 0:1, None].to_broadcast([128, TT, H]))
            for tt in range(TT):
                pt = psB()
                for ki in range(DT):
                    nc.tensor.matmul(pt[:],
                                     src_T[:, ki, tt * 128:(tt + 1) * 128],
                                     w[:, ki], start=(ki == 0),
                                     stop=(ki == DT - 1))
                dstv = v_tok[:, tt].rearrange("p (h c) -> p h c", c=65)[:, :, 0:64]
                nc.vector.tensor_tensor(
                    dstv, pt[:].rearrange("p (h c) -> p h c", c=64),
                    bv_full[bname][:].rearrange("p (h c) -> p h c", c=64),
                    OP.add)

        def attention(q_T, k_T, v_tok, attn_T, causal, after_pair=None):
            """PV with a ones-column appended to V, so the softmax denominator
            lands in psum row 64; normalize via fast reciprocal + broadcast.
            after_pair(hp) interleaves follow-up work (the residual output
            projection block that consumes this pair's heads)."""
            for hp in range(H // 2):
                # the two heads of a pair sit on complementary partition
                # halves (rows 0-63 / 64-127), so their K=64 score matmuls
                # pack into disjoint PE row-groups and run concurrently
                pv = {sub: {qh: psB() for qh in range(TH)} for sub in range(2)}
                for kt in range(TT):
                    q0 = kt * 128 if causal else 0
                    if q0 < 512:
                        chunks = [(q0, 512 - q0), (512, 512)]
                    else:
                        chunks = [(q0, 1024 - q0)]
                    pr = {}
                    for sub in range(2):
                        h = hp * 2 + sub
                        dt_, base = h // 2, (h % 2) * 64
                        q_h = q_T[base:base + 64, dt_]
                        k_h = k_T[base:base + 64, dt_]
                        pt = ps_sc.tile([128, 1024], F32, tag="SC", name="pSC")
                        for (c0, cw) in chunks:
                            nc.tensor.matmul(pt[:, c0:c0 + cw],
                                             k_h[:, kt * 128:(kt + 1) * 128],
                                             q_h[:, c0:c0 + cw],
                                             start=True, stop=True)
                        pr[sub] = cp.tile([128, S], BF16, tag="probs",
                                          name="probs")
                        nc.scalar.activation(pr[sub][:, q0:S], pt[:, q0:S],
                                             AF.Exp, scale=1.0 / 32.0)
                        if causal:
                            # NOT gpsimd: mixing op types there forces an
                            # ~8us ucode LOAD_LIB swap per switch
                            nc.vector.tensor_tensor(pr[sub][:, q0:q0 + 128],
                                                    pr[sub][:, q0:q0 + 128],
                                                    mask_b[:], OP.mult)
                    for sub in range(2):
                        h = hp * 2 + sub
                        v_h = v_tok[:, kt, h * 65:(h + 1) * 65]
                        for qh in range(TH):
                            if causal and qh == 0 and kt > 3:
                                continue
                            off = max(0, q0 - qh * 512)
                            if causal:
                                last = (kt == 3) if qh == 0 else (kt == TT - 1)
                            else:
                                last = (kt == TT - 1)
                            nc.tensor.matmul(
                                pv[sub][qh][:65, off:512], v_h,
                                pr[sub][:, qh * 512 + off:(qh + 1) * 512],
                                start=(kt == 0), stop=last)
                # normalize: rec = 1/denominator (ACT table Reciprocal),
                # broadcast over the 64 head rows, scale raw PV into attn_T
                for sub in range(2):
                    h = hp * 2 + sub
                    dt_, base = h // 2, (h % 2) * 64
                    for qh in range(TH):
                        # move the denominator row to partition 0 first: the
                        # custom-DVE reciprocal and the gpsimd broadcast both
                        # misbehave on HW when sourced from partition 64
                        den = scr.tile([1, 512], F32, tag="den", name="den")
                        nc.vector.tensor_copy(den[:], pv[sub][qh][64:65, :])
                        rec = scr.tile([1, 512], F32, tag="rec", name="rec")
                        nc.vector.reciprocal_approx_fast(rec[:], den[:])
                        rb = scr.tile([64, 512], F32, tag="rb", name="rb")
                        nc.gpsimd.partition_broadcast(rb[:], rec[:])
                        nc.vector.tensor_tensor(
                            attn_T[base:base + 64, dt_,
                                   qh * 512:(qh + 1) * 512],
                            pv[sub][qh][0:64, :], rb[:], OP.mult)
                if after_pair is not None:
                    after_pair(hp)

        def residual_block(wname, src_T, bias_tile, res_T, out_T, stats_sb,
                           dd):
            """out_T[:,dd] = (W^T @ src_T)[dd] + bias + res_T[dd]; stats."""
            w = wT[wname]
            for th in range(TH):
                pt = psB()
                for ki in range(DT):
                    nc.tensor.matmul(
                        pt[:], w[:, dd * 4 + ki, :],
                        src_T[:, ki, th * 512:(th + 1) * 512],
                        start=(ki == 0), stop=(ki == DT - 1))
                dst = out_T[:, dd, th * 512:(th + 1) * 512]
                c = dd * TH + th
                nc.vector.scalar_tensor_tensor(
                    dst, pt[:], bias_tile[:, dd:dd + 1],
                    res_T[:, dd, th * 512:(th + 1) * 512],
                    OP.add, OP.add, accum_out=stats_sb[:, c:c + 1])
                sq = scr.tile([128, 512], F32, tag="sq", name="sq")
                nc.vector.scalar_tensor_tensor(
                    sq[:], dst, 0.0, dst, OP.add, OP.mult,
                    accum_out=stats_sb[:, 8 + c:8 + c + 1])

        def stats_allreduce(stats_sb, name):
            pt = psB()
            nc.tensor.matmul(pt[:1, :16], onesf[:], stats_sb[:],
                             start=True, stop=True)
            red = sb.tile([1, 8], F32, tag=f"red_{name}")
            nc.vector.reduce_sum(red[:, 0:1], pt[0:1, 0:8],
                                 axis=mybir.AxisListType.X)
            nc.vector.reduce_sum(red[:, 1:2], pt[0:1, 8:16],
                                 axis=mybir.AxisListType.X)
            nc.vector.memset(red[:, 2:8], 0.0)
            if collectives:
                ar_in = dram.tile([1, 8], F32, tag=f"ar_in_{name}")
                ar_out = dram.tile([1, 8], F32, tag=f"ar_out_{name}")
                nc.sync.dma_start(ar_in[:], red[:])
                nc.gpsimd.collective_compute(
                    "AllReduce", OP.add,
                    replica_groups=[list(range(N_CORES))],
                    ins=[ar_in.opt()], outs=[ar_out.opt()])
                g = sb.tile([1, 8], F32, tag=f"g_{name}")
                nc.sync.dma_start(g[:], ar_out[:])
            else:
                # single-core build: norm over the local batch only
                g = red
            ntot = NTOT if collectives else float(S * D)
            mu = sb.tile([1, 1], F32, tag=f"mu_{name}")
            nc.vector.tensor_scalar_mul(mu[:], g[:, 0:1], 1.0 / ntot)
            ex2 = sb.tile([1, 1], F32, tag=f"ex2_{name}")
            nc.vector.tensor_scalar_mul(ex2[:], g[:, 1:2], 1.0 / ntot)
            mu2 = sb.tile([1, 1], F32, tag=f"mu2_{name}")
            nc.vector.tensor_tensor(mu2[:], mu[:], mu[:], OP.mult)
            var = sb.tile([1, 1], F32, tag=f"var_{name}")
            nc.vector.tensor_tensor(var[:], ex2[:], mu2[:], OP.subtract)
            epst = sb.tile([1, 1], F32, tag=f"eps_{name}")
            nc.vector.memset(epst[:], EPS)
            std = sb.tile([1, 1], F32, tag=f"std_{name}")
            nc.scalar.activation(std[:], var[:], AF.Sqrt, bias=epst[:])
            rstd = sb.tile([1, 1], F32, tag=f"rstd_{name}")
            nc.vector.reciprocal(rstd[:], std[:])
            nmr = sb.tile([1, 1], F32, tag=f"nmr_{name}")
            nc.vector.tensor_tensor(nmr[:], mu[:], rstd[:], OP.mult)
            nc.vector.tensor_scalar_mul(nmr[:], nmr[:], -1.0)
            rstd_bc = sb.tile([128, 1], F32, tag=f"rstd_bc_{name}")
            nc.gpsimd.partition_broadcast(rstd_bc[:], rstd[:])
            nmr_bc = sb.tile([128, 1], F32, tag=f"nmr_bc_{name}")
            nc.gpsimd.partition_broadcast(nmr_bc[:], nmr[:])
            return rstd_bc, nmr_bc

        def materialize_norm(src_T, dst_T, rstd_bc, nmr_bc):
            for dd in range(DT):
                nc.vector.scalar_tensor_tensor(
                    dst_T[:, dd], src_T[:, dd], rstd_bc[:],
                    nmr_bc[:, :].to_broadcast([128, S]), OP.mult, OP.add)

        # ================= Phase 1: self attention =================
        q_T = sb.tile([128, DT, S], BF16, tag="g_q")
        k_T = sb.tile([128, DT, S], BF16, tag="g_k")
        v_tok = sb.tile([128, TT, H * 65], BF16, tag="g_v")
        attn_T = sb.tile([128, DT, S], BF16, tag="g_attn")
        k2_T = sb.tile([128, DT, S], BF16, tag="g_k2")
        v2_tok = sb.tile([128, TT, H * 65], BF16, tag="g_v2")

        def fill_cross(hp):
            """Cross-attn k/v projection blocks for head-pair hp — fills the
            causal attention's pair-boundary PE gaps (enc-only inputs)."""
            w = wT["wk_c"]
            for th in range(TH):
                pt = psB()
                for ki in range(DT):
                    nc.tensor.matmul(
                        pt[:], w[:, hp * 4 + ki, :],
                        enc_T[:, ki, th * 512:(th + 1) * 512],
                        start=(ki == 0), stop=(ki == DT - 1))
                nc.vector.tensor_scalar_add(
                    k2_T[:, hp, th * 512:(th + 1) * 512], pt[:],
                    bias["bk_c"][:, hp:hp + 1])
            wv = wT["wv_c"]
            for tt in (2 * hp, 2 * hp + 1):
                pt = psB()
                for ki in range(DT):
                    nc.tensor.matmul(pt[:],
                                     enc_T[:, ki, tt * 128:(tt + 1) * 128],
                                     wv[:, ki], start=(ki == 0),
                                     stop=(ki == DT - 1))
                dstv = v2_tok[:, tt].rearrange(
                    "p (h c) -> p h c", c=65)[:, :, 0:64]
                nc.vector.tensor_tensor(
                    dstv, pt[:].rearrange("p (h c) -> p h c", c=64),
                    bv_full["bv_c"][:].rearrange("p (h c) -> p h c", c=64),
                    OP.add)

        project("wq_m", x_T, q_T, bname="bq_m")
        project("wk_m", x_T, k_T, bname="bk_m")
        project_v("wv_m", "bv_m", x_T, v_tok)
        # prep for residual-1 and phase 2 rides behind phase-1 issue
        prep_act("encoder_out", enc_T)
        prep_w("wo_m")
        prep_w("wq_c", colsum_to=wsum_qc)
        prep_w("wk_c")
        prep_wv("wv_c")
        prep_w("wo_c")
        r1_T = sb.tile([128, DT, S], BF16, tag="g_r1")
        stats1 = sb.tile([128, 16], F32, tag="stats1")
        ones2 = v2_tok[:, :, :].rearrange(
            "p t (h c) -> p t h c", c=65)[:, :, :, 64]
        nc.vector.tensor_copy(
            ones2, onesf[:, 0:1, None].to_broadcast([128, TT, H]))
        attention(q_T, k_T, v_tok, attn_T, causal=True,
                  after_pair=fill_cross)
        for dd in range(DT):
            residual_block("wo_m", attn_T, bias["bo_m"], x_T, r1_T, stats1, dd)
        rstd1, nmr1 = stats_allreduce(stats1, "n1")

        # ================= Phase 2: cross attention =================
        # k_c/v_c were computed inside attention-1 (fill_cross). q_c's raw
        # matmuls spread over both psum pools so all 32 can run while AR1 is
        # in flight; only the ACT fixes wait on rstd1.
        qfix = sb.tile([128, DT], F32, tag="qfix")
        for dd in range(DT):
            nc.vector.scalar_tensor_tensor(
                qfix[:, dd:dd + 1], wsum_qc[:, dd:dd + 1], nmr1[:],
                bias["bq_c"][:, dd:dd + 1], OP.mult, OP.add)
        wqc = wT["wq_c"]
        sc = None
        for i, (dd, th) in enumerate((d, t) for d in range(DT)
                                     for t in range(TH)):
            if i % 2 == 0:
                region = psB()[:]
            else:
                if i % 4 == 1:
                    sc = ps_sc.tile([128, 1024], F32, tag="SC", name="pSC")
                region = sc[:, 512:1024] if i % 4 == 3 else sc[:, 0:512]
            for ki in range(DT):
                nc.tensor.matmul(
                    region, wqc[:, dd * 4 + ki, :],
                    r1_T[:, ki, th * 512:(th + 1) * 512],
                    start=(ki == 0), stop=(ki == DT - 1))
            nc.scalar.activation(q_T[:, dd, th * 512:(th + 1) * 512], region,
                                 AF.Identity, bias=qfix[:, dd:dd + 1],
                                 scale=rstd1[:])
        prep_wf()  # FFN weight prep rides behind phase-2 issue
        nmm_T = sb.tile([128, DT, S], BF16, tag="g_x")   # reuses x_T space
        materialize_norm(r1_T, nmm_T, rstd1, nmr1)
        r2_T = sb.tile([128, DT, S], BF16, tag="g_enc")  # reuses enc_T space
        stats2 = sb.tile([128, 16], F32, tag="stats2")
        attention(q_T, k2_T, v2_tok, attn_T, causal=False)
        for dd in range(DT):
            residual_block("wo_c", attn_T, bias["bo_c"], nmm_T, r2_T, stats2,
                           dd)
        rstd2, nmr2 = stats_allreduce(stats2, "n2")

        # ================= Phase 3: FFN =================
        ffix = sb.tile([128, FT], F32, tag="ffix")
        for ft in range(FT):
            nc.vector.scalar_tensor_tensor(
                ffix[:, ft:ft + 1], wsum_f1[:, ft:ft + 1], nmr2[:],
                bf1_sb[:, ft:ft + 1], OP.mult, OP.add)
        nmh_T = sb.tile([128, DT, S], BF16, tag="g_r1")  # reuses r1_T space
        materialize_norm(r2_T, nmh_T, rstd2, nmr2)

        r3_T = sb.tile([128, DT, S], BF16, tag="g_attn")  # reuses attn space
        stats3 = sb.tile([128, 16], F32, tag="stats3")
        h_T = sb.tile([128, FT, 512], BF16, tag="h_T")
        for th in range(TH):
            sc = None
            for ft in range(FT):
                # alternate psum pools: the relu drains wait on AllReduce-2,
                # so spreading over 8 banks lets more FFN1 matmuls run ahead
                if ft % 2 == 0:
                    region = psB()[:]
                else:
                    if ft % 4 == 1:
                        sc = ps_sc.tile([128, 1024], F32, tag="SC", name="pSC")
                    region = sc[:, (ft % 4 == 3) * 512:][:, :512]
                for ki in range(DT):
                    nc.tensor.matmul(
                        region, wf1T[:, ft * 4 + ki, :],
                        r2_T[:, ki, th * 512:(th + 1) * 512],
                        start=(ki == 0), stop=(ki == DT - 1))
                nc.scalar.activation(h_T[:, ft], region, AF.Relu,
                                     bias=ffix[:, ft:ft + 1], scale=rstd2[:])
            for dd in range(DT):
                pt = psB()
                for ki in range(FT):
                    nc.tensor.matmul(
                        pt[:], wf2T[:, dd * 16 + ki, :],
                        h_T[:, ki], start=(ki == 0), stop=(ki == FT - 1))
                dst = r3_T[:, dd, th * 512:(th + 1) * 512]
                c = dd * TH + th
                nc.vector.scalar_tensor_tensor(
                    dst, pt[:], bias["bf2"][:, dd:dd + 1],
                    nmh_T[:, dd, th * 512:(th + 1) * 512], OP.add, OP.add,
                    accum_out=stats3[:, c:c + 1])
                sq = scr.tile([128, 512], F32, tag="sq", name="sq")
                nc.vector.scalar_tensor_tensor(
                    sq[:], dst, 0.0, dst, OP.add, OP.mult,
                    accum_out=stats3[:, 8 + c:8 + c + 1])

        # transpose r3 to token-major via DMA xbar; th=0 halves can go while
        # FFN th=1 still runs, the rest overlaps AllReduce #3
        r3_tok = sb.tile([128, TT, D], BF16, tag="g_q")  # reuses q space
        for th in range(TH):
            for dd in range(DT):
                nc.scalar.dma_start(
                    r3_tok[:, th * 4:(th + 1) * 4, dd * 128:(dd + 1) * 128],
                    r3_T[:, dd, th * 512:(th + 1) * 512], transpose=True)

        rstd3, nmr3 = stats_allreduce(stats3, "n3")
        out_stage = sb.tile([128, TT, D], F32, tag="h_T")  # reuses h_T space
        for tt in range(TT):
            nc.vector.scalar_tensor_tensor(
                out_stage[:, tt], r3_tok[:, tt], rstd3[:],
                nmr3[:, :].to_broadcast([128, D]), OP.mult, OP.add)
            nc.scalar.dma_start(
                out_d.rearrange("(tt p) d -> p tt d", p=128)[:, tt],
                out_stage[:, tt])


_NC_CACHE = {}


def make_in_maps(inputs):
    in_maps = []
    for b in range(N_CORES):
        m = {"data_dec": np.ascontiguousarray(
                 np.asarray(inputs["data_dec"], dtype=np.float32)[b]),
             "encoder_out": np.ascontiguousarray(
                 np.asarray(inputs["encoder_out"], dtype=np.float32)[b])}
        for k, v in inputs.items():
            if k not in ("data_dec", "encoder_out"):
                m[k] = np.ascontiguousarray(np.asarray(v, dtype=np.float32))
        in_maps.append(m)
    return in_maps


def kernel(**inputs):
    if "nc" not in _NC_CACHE:
        _NC_CACHE["nc"] = build_nc()
    nc = _NC_CACHE["nc"]
    res = bass_utils.run_bass_kernel_spmd(nc, make_in_maps(inputs),
                                          core_ids=list(range(N_CORES)))
    return np.stack([res.results[b]["out"] for b in range(N_CORES)], axis=0)
